# revision 1
# baseline (speedup 1.0000x reference)
"""Distributed kNN OOD-score kernel for 8 Trainium2 NeuronCores.

Problem: for each of 4*32*32 query vectors (D=768), find the 3 nearest
database vectors (N=20000, squared-L2), average the 3 distances, and
bilinearly upsample the resulting [4,32,32] map to [4,1,512,512].

Sharding: queries are data-parallel. Each core owns half of one batch
image (16 of 32 query rows = 512 queries); the database is replicated
and streamed through SBUF in fp8 (e4m3). The one halo row each core
needs for the 16x bilinear upsample is exchanged with its pair core via
a tiny AllGather, scheduled early (boundary query tile first, its last
two super-chunks processed ahead of the other tiles) so its ~15us
latency hides under the remaining compute stream.

Per-core device program (v4, fp8 DoubleRow + full PE-side max-fold):
  - every database column PAIR (x0, x1) is pre-folded on the host into
    a=(x0+x1)/2 and b=(x0-x1)/2 streams. The device computes
        u = q.a + xh_a   and   v = q.b + xh_b
    (so u,v = (t0+-t1)/2 for scores t = q.x - ||x||^2/2), ScalarE takes
    |v| to SBUF bf16, and TensorE adds it onto u with a bf16 identity
    matmul: u + |v| = max(t0, t1) exactly. VectorE then scans only
    1000 folded values per 2000-col half-strip.
  - the cross products keep 764 of 768 dims; the 4 freed contraction
    rows carry xh_* as a 4-level fp8 split (x4 stationary scale baked
    into constant query rows), so scoring is EXACTLY 3 fp8 e4m3
    DoubleRow matmuls per bank (0.5 cyc/col each). ||q||^2 and ||x||^2
    stay exact over all 768 dims; the dropped cross terms add ~4 rms
    on d^2 ~ 1536 (top-3 spacing ~20) -- well inside the tolerance.
  - per query-tile: max8 over the 10 strip top-8s -> top-3, then
    mean distance = reduce_sum of sqrt((q^2 - 2t)/9); ood values are
    transposed into map layout with 4 tiny selector matmuls on the PE
    (no DRAM round-trip).
  - pair AllGather of the boundary tile's 128 ood values (bf16).
  - 16x bilinear upsample = two small bf16 matmuls (interp weights are
    odd/32 fractions - exact in bf16; verified vs jax.image.resize).
"""

import sys

if "/opt/trn_rl_repo" not in sys.path:
    sys.path.insert(0, "/opt/trn_rl_repo")

import numpy as np
import ml_dtypes

import concourse.bass as bass
import concourse.bacc as bacc
import concourse.mybir as mybir
import concourse.tile as tile
from concourse import bass_utils

# Problem shape (hardcoded per contract).
B, D, H, W = 4, 768, 32, 32
N = 20000
K_NN = 3
OUT_H = OUT_W = 512
N_CORES = 8

SC = 4000            # db columns per super-chunk
N_SC = N // SC       # 5
N_HS = 2 * N_SC      # 2000-col half-strips per query tile
QPC = 512            # queries scored per core (16 rows)
N_QT = QPC // 128    # 4
NKP = 3              # K pairs: 764 data dims + 4 xh rows = 3 * (2*128)
DX = 764             # cross-term dims (768 minus 4 freed for xh rows)
OROWS = 256          # output rows per core
NCOL = 24            # ood columns entering the upsample (16 own + 2x4 gathered)
XS = 4.0             # xh scale, baked into constant query rows (fp8 range fit)
DEBUG = False        # adds intermediate-tensor outputs for debugging

F32 = mybir.dt.float32
BF16 = mybir.dt.bfloat16
FP8 = mybir.dt.float8e4
AX = mybir.AxisListType
AF = mybir.ActivationFunctionType
DR = mybir.MatmulPerfMode.DoubleRow

# local tile -> 4-row block of this core's half (block i = rows 4i..4i+3).
# Tile 0 is the block the PAIR core needs as its halo row: for the top
# half (rows 0-15) that's block 3 (row 15), for the bottom half (rows
# 16-31) block 0 (row 16).
TILE_BLOCKS = ([3, 0, 1, 2], [0, 1, 2, 3])

# Strip processing order: boundary query tile (qt 0) gets its last two
# super-chunks early so its AllGather launches ~15us before the stream
# ends.
STRIPS = (
    [(sc, qt) for sc in range(3) for qt in range(N_QT)]
    + [(3, 0), (4, 0)]
    + [(3, qt) for qt in range(1, N_QT)]
    + [(4, qt) for qt in range(1, N_QT)]
)


def _build_program():
    nc = bacc.Bacc(
        "TRN2", target_bir_lowering=False, debug=False, num_devices=N_CORES
    )
    q8d = nc.dram_tensor("q8", [128, NKP, 2, QPC], FP8, kind="ExternalInput").ap()
    db8d = nc.dram_tensor("db8", [128, NKP, 2, N], FP8, kind="ExternalInput").ap()
    q2d = nc.dram_tensor("q2", [128, N_QT], F32, kind="ExternalInput").ap()
    identd = nc.dram_tensor("ident", [128, 2, 128], FP8, kind="ExternalInput").ap()
    s4d = nc.dram_tensor("s4", [128, 4, W], BF16, kind="ExternalInput").ap()
    artd = nc.dram_tensor("art", [NCOL, OROWS], BF16, kind="ExternalInput").ap()
    acd = nc.dram_tensor("ac", [W, OUT_W], BF16, kind="ExternalInput").ap()
    out = nc.dram_tensor("out", [OROWS, OUT_W], F32, kind="ExternalOutput").ap()
    if DEBUG:
        dbg_parts = nc.dram_tensor(
            "dbg_parts", [128, N_HS * 16], F32, kind="ExternalOutput"
        ).ap()
        dbg_oodht = nc.dram_tensor(
            "dbg_oodht", [W, NCOL], BF16, kind="ExternalOutput"
        ).ap()

    with tile.TileContext(nc) as tc:
        with (
            tc.tile_pool(name="static", bufs=1) as sp,
            tc.tile_pool(name="dbh", bufs=6) as dbhp,
            tc.tile_pool(name="db", bufs=9) as dbp,
            tc.tile_pool(name="absv", bufs=5) as avp,
            tc.tile_pool(name="small", bufs=4) as smp,
            tc.tile_pool(name="psum", bufs=4, space="PSUM") as pp,
            tc.tile_pool(name="dram", bufs=1, space="DRAM") as dp,
        ):
            # queries first (needed by the very first matmul)
            q8 = sp.tile([128, NKP, 2, QPC], FP8)
            nc.sync.dma_start(q8[:], q8d[:])

            # super-chunk 0 as 2-col-half tiles for fast startup (the first
            # half of each arrives as two 1000-slot DMAs so the very first
            # quarter's matmuls start ~2us earlier); sc 1..4 as full
            # [128, 2, 4000] tiles per K-pair.
            db0 = {}  # (kp, h) -> tile  (h: slots h*2000..h*2000+2000)
            for h in range(2):
                for kp in range(NKP):
                    db0[(kp, h)] = dbhp.tile(
                        [128, 2, 2000], FP8, tag="dbh", name=f"db0_{kp}_{h}"
                    )
            for qtr in range(4):
                for kp in range(NKP):
                    h, off = divmod(qtr * 1000, 2000)
                    nc.sync.dma_start(
                        db0[(kp, h)][:, :, off : off + 1000],
                        db8d[:, kp, :, qtr * 1000 : (qtr + 1) * 1000],
                    )
            q2_sb = sp.tile([128, N_QT], F32)
            nc.sync.dma_start(q2_sb[:], q2d[:])
            ident = sp.tile([128, 2, 128], FP8)
            nc.sync.dma_start(ident[:], identd[:])
            s4 = sp.tile([128, 4, W], BF16)
            nc.sync.dma_start(s4[:], s4d[:])
            dbt = {}  # sc -> [kp] tiles
            for sc in range(1, N_SC):
                tiles = []
                for kp in range(NKP):
                    t = dbp.tile([128, 2, SC], FP8, tag="db", name=f"db{sc}_{kp}")
                    nc.sync.dma_start(
                        t[:], db8d[:, kp, :, sc * SC : (sc + 1) * SC]
                    )
                    tiles.append(t)
                dbt[sc] = tiles
            # upsample operands, needed only at the very end
            art_sb = sp.tile([NCOL, OROWS], BF16)
            nc.sync.dma_start(art_sb[:], artd[:])
            ac_sb = sp.tile([W, OUT_W], BF16)
            nc.sync.dma_start(ac_sb[:], acd[:])

            # per-query-tile top-8 of each quarter-strip's folded maxima
            parts = [
                sp.tile([128, N_HS * 16], F32, name=f"part{qt}")
                for qt in range(N_QT)
            ]
            oods = [
                sp.tile([128, 1], BF16, name=f"ood{qt}") for qt in range(N_QT)
            ]
            cc_in = dp.tile([128], BF16)
            cc_out = dp.tile([256], BF16)
            # ood_hT[c, j]: j 0..15 own rows (local order), 16..23 the two
            # gathered boundary blocks in rank order; filled incrementally
            # as each query tile finishes
            ood_hT = sp.tile([W, NCOL], BF16)

            def rhs(sc, kp, col, width):
                """db slots [col, col+width) of super-chunk sc, K-pair kp."""
                if sc == 0:
                    h, off = divmod(col, 2000)
                    return db0[(kp, h)][:, :, off : off + width]
                return dbt[sc][kp][:, :, col : col + width]

            def qt_epilogue(qt):
                f8 = smp.tile([128, 8], F32, tag="f8", name=f"f8_{qt}")
                nc.vector.max(f8[:], parts[qt][:])
                # dist_j/3 = sqrt((q2 - 2 t_j) / 9); host passes q2/9
                d3 = smp.tile([128, K_NN], F32, tag="d3", name=f"d3_{qt}")
                nc.scalar.activation(
                    d3[:],
                    f8[:, 0:K_NN],
                    AF.Sqrt,
                    bias=q2_sb[:, qt : qt + 1],
                    scale=-2.0 / 9.0,
                )
                with nc.allow_low_precision(
                    reason="3-element sum rounded to bf16 for the ood "
                    "exchange; ~0.2% on a 2% tolerance"
                ):
                    nc.vector.reduce_sum(oods[qt][:], d3[:], axis=AX.X)
                if qt == 0:
                    # boundary block: gather it across the pair ASAP so
                    # the ~15us collective hides under remaining work
                    nc.sync.dma_start(cc_in[:], oods[0][:])
                    nc.gpsimd.collective_compute(
                        "AllGather",
                        mybir.AluOpType.bypass,
                        replica_groups=[[0, 1], [2, 3], [4, 5], [6, 7]],
                        ins=[cc_in.opt()],
                        outs=[cc_out.opt()],
                    )
                    nc.sync.dma_start(
                        ood_hT[:, 16:NCOL],
                        cc_out.rearrange("(b r c) -> c (b r)", b=2, c=W),
                    )
                # the [128,1] -> [32,4] transpose into ood_hT is deferred
                # to after the strips loop: its selector matmuls would
                # otherwise block the in-order PE queue on this reduce

            # deferred per-half-strip work: (u_tile, absv_tile, part_ap)
            pending = []

            def drain_one():
                u, absv, part_ap = pending.pop(0)
                # u += |v| closes the folded accumulation group (exact
                # max up to fp8 rounding of |v|: u + |v| = max(t0, t1)).
                # DoubleRow at 0.5 cyc/col: stationary k-tile0 = identity,
                # k-tile1 = zeros; absv's k-tile1 is memset to 0 once.
                nc.tensor.matmul(
                    u[:, 0, 0:500],
                    ident[:],
                    absv[:, :, 0:500],
                    start=False,
                    stop=True,
                    perf_mode=DR,
                )
                nc.vector.max(part_ap, u[:, 0, 0:500])

            for si, (sc, qt) in enumerate(STRIPS):
                lhsT = [
                    q8[:, kp, :, qt * 128 : (qt + 1) * 128] for kp in range(NKP)
                ]
                for qr in range(4):
                    g0 = qr * 1000          # in-chunk slot base
                    # a-slots [g0, g0+500) -> u bank,
                    # b-slots [g0+500, g0+1000) -> v bank; 1-bank tiles in
                    # a 4-deep rotation so the deferred id-add + max8 never
                    # stall the PE on PSUM reuse
                    u_ps = pp.tile([128, 1, 512], F32, tag="u", name="u", bufs=4)
                    v_ps = pp.tile([128, 1, 512], F32, tag="v", name="v", bufs=4)
                    for kp in range(NKP):
                        nc.tensor.matmul(
                            u_ps[:, 0, 0:500],
                            lhsT[kp],
                            rhs(sc, kp, g0, 500),
                            start=(kp == 0),
                            stop=False,
                            perf_mode=DR,
                        )
                        nc.tensor.matmul(
                            v_ps[:, 0, 0:500],
                            lhsT[kp],
                            rhs(sc, kp, g0 + 500, 500),
                            start=(kp == 0),
                            stop=(kp == NKP - 1),
                            perf_mode=DR,
                        )
                    # ScalarE: |v| -> SBUF fp8 (k-tile1 zeroed once so the
                    # DoubleRow id-add's second lane contributes nothing)
                    absv = avp.tile([128, 2, 500], FP8, tag="absv", name="absv")
                    if si * 4 + qr < 5:
                        nc.gpsimd.memset(absv[:, 1, 0:500], 0.0)
                    nc.scalar.activation(
                        absv[:, 0, 0:500], v_ps[:, 0, 0:500], AF.Abs
                    )
                    pending.append(
                        (u_ps, absv, parts[qt][:, (sc * 4 + qr) * 8 :][:, 0:8])
                    )
                    # drain the 2-quarters-old id-add + max8 here: by then
                    # its |v| activation has long finished, so neither PE
                    # nor DVE stalls on the ScalarE chain
                    while len(pending) > 3:
                        drain_one()

                is_qt_last = (sc, qt) in ((4, 0), (4, 1), (4, 2), (4, 3))
                if is_qt_last:
                    while pending:
                        drain_one()
                    qt_epilogue(qt)

            # own blocks into the upsample operand: the PE transposes each
            # [128,1] ood vector into [32,4] with 4 per-row-block selector
            # matmuls (S4[:,b,:].T @ ood picks partitions b*32..b*32+31),
            # then ScalarE drops it into ood_hT -- much lower latency than
            # a DMA round-trip through DRAM, and emitted here so only the
            # last query tile's transpose is on the critical path
            for qt in range(N_QT):
                oht_ps = pp.tile([W, 4], F32, tag="u", name=f"oht{qt}", bufs=4)
                for blk in range(4):
                    nc.tensor.matmul(
                        oht_ps[:, blk : blk + 1],
                        s4[:, blk, :],
                        oods[qt][:],
                        start=True,
                        stop=True,
                    )
                nc.scalar.activation(
                    ood_hT[:, qt * 4 : (qt + 1) * 4], oht_ps[:], AF.Copy
                )

            if DEBUG:
                nc.sync.dma_start(dbg_parts[:], parts[0][:])
                nc.sync.dma_start(dbg_oodht[:], ood_hT[:])

            # P1[j, ow] = sum_c ood_hT[c, j] * A_c[c, ow]
            p1 = pp.tile([NCOL, OUT_W], F32, tag="v", name="p1", bufs=4)
            nc.tensor.matmul(p1[:], ood_hT[:], ac_sb[:], start=True, stop=True)
            p1_sb = sp.tile([NCOL, OUT_W], BF16)
            nc.scalar.activation(p1_sb[:], p1[:], AF.Copy)
            # out[oi, ow] = sum_j art[j, oi] * P1[j, ow]
            for m in range(2):
                p2 = pp.tile([128, OUT_W], F32, tag="v", name=f"p2_{m}", bufs=4)
                nc.tensor.matmul(
                    p2[:],
                    art_sb[:, m * 128 : (m + 1) * 128],
                    p1_sb[:],
                    start=True,
                    stop=True,
                )
                o_sb = smp.tile([128, OUT_W], F32, tag="osb", name=f"osb{m}")
                nc.scalar.activation(o_sb[:], p2[:], AF.Copy)
                nc.sync.dma_start(out[m * 128 : (m + 1) * 128, :], o_sb[:])

    nc.compile()
    return nc


def _bilinear_matrix(out_size: int, in_size: int) -> np.ndarray:
    """Half-pixel (align_corners=False) bilinear interpolation matrix
    [out_size, in_size]; edge-clamped, equivalent to jax.image.resize
    'bilinear' for integer upsampling."""
    A = np.zeros((out_size, in_size), dtype=np.float64)
    scale = in_size / out_size
    for i in range(out_size):
        s = (i + 0.5) * scale - 0.5
        j0 = int(np.floor(s))
        w = s - j0
        A[i, min(max(j0, 0), in_size - 1)] += 1.0 - w
        A[i, min(max(j0 + 1, 0), in_size - 1)] += w
    return A.astype(np.float32)


_NC_CACHE = None


def _get_nc():
    global _NC_CACHE
    if _NC_CACHE is None:
        _NC_CACHE = _build_program()
    return _NC_CACHE


def _slot_pack(database: np.ndarray):
    """Fold all column pairs: per 1000-col quarter-strip the slot layout
    is [500 a=(x0+x1)/2 | 500 b=(x0-x1)/2] over 764 dims, with the
    matching xh = -(||x0||^2 +- ||x1||^2)/4 terms (over all 768 dims,
    scaled 1/XS) as a 4-level fp8 split in rows 764..767."""
    h = 0.5 * np.einsum("nd,nd->n", database, database)    # ||x||^2/2
    dbX = np.empty((N, 768), dtype=np.float32)             # slot-major
    xhX = np.empty(N, dtype=np.float32)
    for g in range(N // 1000):
        base = g * 1000
        p0 = database[base : base + 1000 : 2, :DX]
        p1 = database[base + 1 : base + 1000 : 2, :DX]
        h0 = h[base : base + 1000 : 2]
        h1 = h[base + 1 : base + 1000 : 2]
        dbX[base : base + 500, :DX] = 0.5 * (p0 + p1)
        dbX[base + 500 : base + 1000, :DX] = 0.5 * (p0 - p1)
        xhX[base : base + 500] = -0.5 * (h0 + h1)
        xhX[base + 500 : base + 1000] = -0.5 * (h0 - h1)
    # 4-level fp8 split of xh/XS into the 4 spare contraction rows
    r = (xhX / XS).astype(np.float32)
    for lv in range(4):
        q = r.astype(ml_dtypes.float8_e4m3).astype(np.float32)
        dbX[:, DX + lv] = q
        r = r - q
    return dbX


def make_in_maps(embeddings: np.ndarray, database: np.ndarray):
    embeddings = np.asarray(embeddings, dtype=np.float32)
    database = np.asarray(database, dtype=np.float32)

    dbX = _slot_pack(database)
    # contraction-pair layout: db8[p, kp, i, n] = dbX[n, kp*256+i*128+p]
    db8 = np.ascontiguousarray(
        dbX.T.reshape(NKP, 2, 128, N).transpose(2, 0, 1, 3)
    ).astype(ml_dtypes.float8_e4m3)

    q_all = embeddings.transpose(0, 2, 3, 1).reshape(B, H * W, D)
    Ac = _bilinear_matrix(OUT_W, W)                      # [512, 32]
    Ar = _bilinear_matrix(OUT_H, H)                      # [512, 32]
    # DoubleRow identity: k-tile0 = I (adds |v|), k-tile1 = 0
    ident = np.zeros((128, 2, 128), dtype=np.float32)
    ident[:, 0, :] = np.eye(128, dtype=np.float32)
    ident = ident.astype(ml_dtypes.float8_e4m3)
    # s4[q, b, c] = 1 iff q == b*32+c: per-row-block selectors that let the
    # PE transpose a [128,1] ood vector into [32,4] map layout
    s4 = np.eye(128, dtype=np.float32).reshape(128, 4, W).astype(
        ml_dtypes.bfloat16
    )
    # the two gathered blocks in cc_out rank order: pair-core tile 0 rows
    cc_rows = [12, 13, 14, 15, 16, 17, 18, 19]

    in_maps = []
    for c in range(N_CORES):
        b, half = divmod(c, 2)
        blocks = TILE_BLOCKS[half]
        own_rows = [16 * half + 4 * blk + r for blk in blocks for r in range(4)]

        # queries in local-tile order; the 4 spare rows carry the xh
        # stationary scale XS
        q = np.concatenate(
            [
                q_all[b, (16 * half + 4 * blk) * W : (16 * half + 4 * blk + 4) * W]
                for blk in blocks
            ]
        )                                                # [512, 768]
        qX = q.copy()
        qX[:, DX:] = XS
        q8 = np.ascontiguousarray(
            qX.T.reshape(NKP, 2, 128, QPC).transpose(2, 0, 1, 3)
        ).astype(ml_dtypes.float8_e4m3)                  # [128, 3, 2, 512]
        q2 = np.einsum("qd,qd->q", q, q) / 9.0
        q2 = np.ascontiguousarray(q2.reshape(N_QT, 128).T.astype(np.float32))

        # interpolation rows matching ood_hT's column order
        Arh = Ar[half * OROWS : (half + 1) * OROWS]      # [256, 32]
        art = np.zeros((NCOL, OROWS), dtype=np.float32)
        for j, row in enumerate(own_rows):
            art[j] = Arh[:, row]
        for j, row in enumerate(cc_rows):
            if row not in own_rows:
                art[16 + j] = Arh[:, row]
        in_maps.append(
            {
                "db8": db8,
                "q8": q8,
                "q2": q2,
                "ident": ident,
                "s4": s4,
                "art": art.astype(ml_dtypes.bfloat16),
                "ac": np.ascontiguousarray(Ac.T).astype(ml_dtypes.bfloat16),
            }
        )
    return in_maps


def run_device(in_maps, **kwargs):
    nc = _get_nc()
    return bass_utils.run_bass_kernel_spmd(
        nc, in_maps, core_ids=list(range(N_CORES)), **kwargs
    )


def kernel(embeddings, database, k, out_h, out_w):
    assert int(k) == K_NN and int(out_h) == OUT_H and int(out_w) == OUT_W
    in_maps = make_in_maps(np.asarray(embeddings), np.asarray(database))
    res = run_device(in_maps)
    out = np.empty((B, 1, OUT_H, OUT_W), dtype=np.float32)
    for c in range(N_CORES):
        b, half = divmod(c, 2)
        out[b, 0, half * OROWS : (half + 1) * OROWS] = res.results[c]["out"]
    return out



# revision 16
# speedup vs baseline: 1.0716x; 1.0716x over previous
"""Distributed kNN OOD-score kernel for 8 Trainium2 NeuronCores (v5).

Problem: for each of 4*32*32 query vectors (D=768), find the 3 nearest
database vectors (N=20000, squared-L2), average the 3 distances, and
bilinearly upsample the resulting [4,32,32] map to [4,1,512,512].

Sharding (v5, pair-split database): cores 2b and 2b+1 both work on batch
image b. The DATABASE is split between them (core half=0 streams entries
0..9999, half=1 streams 10000..19999 -- 7.7MB fp8 per core instead of
15.4MB replicated, which removes the DMA-starvation the v4 kernel hit),
and each core scores its half against ALL 1024 queries of the batch
(8 query tiles of 128). Per-tile top-8 candidates are then exchanged
within the pair by ONE small AllGather and merged (top-8 of own-24) to
exact top-3 over the full database.

Tile order per core: [P_bnd, P_a, P_b, P_c, M_bnd, M_a, M_b, M_c] where
P_* are the partner's map-row blocks and M_* this core's own. The gather
payload is positions 0-4 (everything the partner needs: its four blocks
+ my boundary block for the bilinear halo), so the collective fires at
~42us and its ~19us latency (15us rendezvous + 2x2.7us DRAM staging)
lands just as the 58us scoring stream ends. The halo ood block is
computed redundantly on both cores from the gathered candidates, so no
second exchange is needed.

Scoring per 1000-col unit (same fp8 DoubleRow scheme as v4, which is at
the PE fp8 peak): db column pairs are host-folded into a=(x0+x1)/2,
b=(x0-x1)/2 streams; 6 DR matmuls give u,v banks; ScalarE takes |v| to
SBUF fp8; one DR identity-matmul adds it back (u+|v| = max(t0,t1)
exactly); DVE max8 scans the 500 folded maxima. The cross products keep
764 of 768 dims; 4 freed contraction rows carry -||x||^2/2 as a 4-level
fp8 split (x4 stationary scale).

Epilogue: per-tile merges -> mean top-3 distance -> PE-transpose into
map layout -> one [32,20]x[32,512] interpolation matmul (p1) -> two
[20,128]x[20,512] matmuls (p2) -> 2x[128,512] output DMAs.
"""

import sys

if "/opt/trn_rl_repo" not in sys.path:
    sys.path.insert(0, "/opt/trn_rl_repo")

import numpy as np
import ml_dtypes

import concourse.bass as bass
import concourse.bacc as bacc
import concourse.mybir as mybir
import concourse.tile as tile
from concourse import bass_utils

# Problem shape (hardcoded per contract).
B, D, H, W = 4, 768, 32, 32
N = 20000
K_NN = 3
OUT_H = OUT_W = 512
N_CORES = 8

NHALF = N // 2        # db entries per core
N_SC = NHALF // 1000  # 10 units of 1000 cols per query tile
NT = 8                # query tiles per core (128 queries each)
QTOT = NT * 128       # 1024 queries scored per core
QW = QTOT + 128       # q8 width: +128 cols carrying the DR identity
QCOL = [0] + [128 * (p + 1) for p in range(1, NT)]  # q8 col of tile p
NKP = 3               # K pairs: 764 data dims + 4 xh rows = 3 * (2*128)
DX = 764              # cross-term dims (768 minus 4 freed for xh rows)
NCOL = 20             # ood columns entering the upsample (16 own + 4 halo)
XS = 4.0              # xh scale, baked into constant query rows
NEG = -1.0e9          # mask value killing own-rank gather blocks

F32 = mybir.dt.float32
BF16 = mybir.dt.bfloat16
FP8 = mybir.dt.float8e4
AX = mybir.AxisListType
AF = mybir.ActivationFunctionType
ALU = mybir.AluOpType
DR = mybir.MatmulPerfMode.DoubleRow

# Map-row start of each 4-row block, per half, in position order
# [P_bnd, P_a, P_b, P_c, M_bnd, M_a, M_b, M_c].
POS_ROWS = (
    [16, 20, 24, 28, 12, 8, 4, 0],   # half 0 (top, own rows 0-15)
    [12, 8, 4, 0, 16, 20, 24, 28],   # half 1 (bottom, own rows 16-31)
)

# Unit schedule: sc-major over payload positions 0-4 first (their last
# chunk-9 units run right after chunk 9 lands at ~25us, so the payload is
# complete at ~39us and the collective hides), then own tiles 5-7.
STRIPS = (
    [(sc, pos) for sc in range(N_SC) for pos in range(5)]
    + [(sc, pos) for pos in range(5, NT) for sc in range(N_SC)]
)
# merge index m: 0=M_bnd, 1=M_a, 2=M_b, 3=M_c, 4=halo(P_bnd)
# q2 column of the tile each merge scores
MERGE_Q2COL = [4, 5, 6, 7, 0]
DEBUG = False


def _build_program():
    nc = bacc.Bacc(
        "TRN2", target_bir_lowering=False, debug=False, num_devices=N_CORES
    )
    q8d = nc.dram_tensor("q8", [128, NKP, 2, QW], FP8, kind="ExternalInput").ap()
    db8d = nc.dram_tensor(
        "db8", [128, NKP, 2, NHALF], FP8, kind="ExternalInput"
    ).ap()
    q2d = nc.dram_tensor("q2", [128, 5, K_NN], F32, kind="ExternalInput").ap()
    s4d = nc.dram_tensor("s4", [128, 4, W], F32, kind="ExternalInput").ap()
    maskd = nc.dram_tensor("mask", [128, 2, 8], F32, kind="ExternalInput").ap()
    artd = nc.dram_tensor("art", [NCOL, 2, 128], BF16, kind="ExternalInput").ap()
    acd = nc.dram_tensor("ac", [W, OUT_W], BF16, kind="ExternalInput").ap()
    out = nc.dram_tensor("out", [2, 128, OUT_W], BF16, kind="ExternalOutput").ap()
    if DEBUG:
        dbg_loc8 = nc.dram_tensor(
            "dbg_loc8", [128, NT, 8], F32, kind="ExternalOutput"
        ).ap()
        dbg_oodht = nc.dram_tensor(
            "dbg_oodht", [W, NCOL], BF16, kind="ExternalOutput"
        ).ap()

    with tile.TileContext(nc) as tc:
        with (
            tc.tile_pool(name="static", bufs=1) as sp,
            tc.tile_pool(name="db", bufs=N_SC * NKP) as dbp,
            tc.tile_pool(name="absv", bufs=5) as avp,
            tc.tile_pool(name="small", bufs=8) as smp,
            tc.tile_pool(name="psum", bufs=4, space="PSUM") as pp,
            tc.tile_pool(name="dram", bufs=1, space="DRAM") as dp,
        ):
            # ---- input DMAs, in first-need order. The SP sequencer holds
            # ~650ns per dma_start, so the count is kept low: 3 q8 pieces
            # (the DR identity rides in q8 cols 128:256), per-kp chunks for
            # sc0 only, whole-chunk DMAs for sc1-9, misc last. ----
            q8 = sp.tile([128, NKP, 2, QW], FP8)
            # pos0 queries first (first matmul), identity block second
            # (first drain, ~3 units later)
            nc.sync.dma_start(q8[:, :, :, 0:128], q8d[:, :, :, 0:128])
            dbt = {}  # sc -> [128, NKP, 2, 1000] tile
            db0 = dbp.tile([128, NKP, 2, 1000], FP8, tag="db", name="db0")
            for kp in range(NKP):
                nc.sync.dma_start(db0[:, kp, :, :], db8d[:, kp, :, 0:1000])
            dbt[0] = db0
            nc.sync.dma_start(q8[:, :, :, 128:256], q8d[:, :, :, 128:256])
            nc.sync.dma_start(q8[:, :, :, 256:768], q8d[:, :, :, 256:768])
            for sc in range(1, N_SC):
                t = dbp.tile([128, NKP, 2, 1000], FP8, tag="db", name=f"db{sc}")
                nc.sync.dma_start(
                    t[:], db8d[:, :, :, sc * 1000 : (sc + 1) * 1000]
                )
                dbt[sc] = t
                if sc == 4:
                    nc.sync.dma_start(q8[:, :, :, 768:QW], q8d[:, :, :, 768:QW])
            q2_sb = sp.tile([128, 5, K_NN], F32)
            nc.sync.dma_start(q2_sb[:], q2d[:])
            ident = q8[:, 0, :, 128:256]
            mask_sb = sp.tile([128, 2, 8], F32)
            nc.sync.dma_start(mask_sb[:], maskd[:])
            s4 = sp.tile([128, 4, W], F32)
            nc.sync.dma_start(s4[:], s4d[:])
            art_sb = sp.tile([NCOL, 2, 128], BF16)
            nc.sync.dma_start(art_sb[:], artd[:])
            ac_sb = sp.tile([W, OUT_W], BF16)
            nc.sync.dma_start(ac_sb[:], acd[:])

            # ---- working state ----
            parts = [
                sp.tile([128, N_SC * 8], F32, name=f"part{p}") for p in range(NT)
            ]
            loc8_pay = sp.tile([128, 5, 8], F32)       # payload top-8s, pos 0-4
            # merge staging: slot0 = own top-8, slots 1-2 = masked gather
            mrg = [sp.tile([128, 3, 8], F32, name=f"mrg{m}") for m in range(5)]
            cc_in = dp.tile([128 * 40], F32)
            cc_out = dp.tile([2 * 128 * 40], F32)
            rem8 = sp.tile([128, 2, 40], F32)          # gathered payloads
            ood_hT = sp.tile([W, NCOL], BF16)

            def rhs(sc, kp, col, width):
                return dbt[sc][:, kp, :, col : col + width]

            pending = []

            def drain_one():
                u, absv, part_ap = pending.pop(0)
                nc.tensor.matmul(
                    u[:, 0, 0:500],
                    ident,
                    absv[:, :, 0:500],
                    start=False,
                    stop=True,
                    perf_mode=DR,
                )
                nc.vector.max(part_ap, u[:, 0, 0:500])

            def tile_epilogue(pos):
                m = pos - 4  # merge index for own tiles (pos 5,6,7 -> 1,2,3)
                if pos <= 4:
                    out8 = loc8_pay[:, pos, :]
                else:
                    out8 = mrg[m][:, 0, :]
                nc.vector.max(out8, parts[pos][:])
                if pos == 4:
                    # payload complete: pair-exchange positions 0-4
                    # (gpsimd-issued DMAs: 25ns sequencer hold vs 650 on SP)
                    nc.gpsimd.dma_start(cc_in[:], loc8_pay[:])
                    nc.gpsimd.collective_compute(
                        "AllGather",
                        ALU.bypass,
                        replica_groups=[[0, 1], [2, 3], [4, 5], [6, 7]],
                        ins=[cc_in.opt()],
                        outs=[cc_out.opt()],
                    )
                    nc.sync.dma_start(
                        rem8[:],
                        cc_out.rearrange("(r p f) -> p r f", r=2, p=128),
                    )

            # last strip index of each tile, and where to emit its epilogue
            # (>=3 units later so the pending drain has naturally passed it)
            last_idx = {}
            for i, (sc, pos) in enumerate(STRIPS):
                last_idx[pos] = i
            emit_at = {}
            for pos, idx in last_idx.items():
                emit_at.setdefault(min(idx + 3, len(STRIPS) - 1), []).append(pos)

            memset_count = 0
            for si, (sc, pos) in enumerate(STRIPS):
                c0 = QCOL[pos]
                lhsT = [
                    q8[:, kp, :, c0 : c0 + 128] for kp in range(NKP)
                ]
                u_ps = pp.tile([128, 1, 512], F32, tag="u", name="u", bufs=4)
                v_ps = pp.tile([128, 1, 512], F32, tag="v", name="v", bufs=4)
                for kp in range(NKP):
                    nc.tensor.matmul(
                        u_ps[:, 0, 0:500],
                        lhsT[kp],
                        rhs(sc, kp, 0, 500),
                        start=(kp == 0),
                        stop=False,
                        perf_mode=DR,
                    )
                    nc.tensor.matmul(
                        v_ps[:, 0, 0:500],
                        lhsT[kp],
                        rhs(sc, kp, 500, 500),
                        start=(kp == 0),
                        stop=(kp == NKP - 1),
                        perf_mode=DR,
                    )
                absv = avp.tile([128, 2, 500], FP8, tag="absv", name="absv")
                if memset_count < 5:
                    nc.gpsimd.memset(absv[:, 1, 0:500], 0.0)
                    memset_count += 1
                nc.scalar.activation(absv[:, 0, 0:500], v_ps[:, 0, 0:500], AF.Abs)
                pending.append((u_ps, absv, parts[pos][:, sc * 8 :][:, 0:8]))
                while len(pending) > 2:
                    drain_one()
                for pos_done in emit_at.get(si, []):
                    if last_idx[pos_done] < si - 2:
                        tile_epilogue(pos_done)
                    else:
                        # too fresh (end of stream): flush pending first
                        while pending:
                            drain_one()
                        tile_epilogue(pos_done)
            while pending:
                drain_one()

            # own-top8 slots for the bnd and halo merges come from the payload
            nc.vector.tensor_copy(mrg[0][:, 0:1, :], loc8_pay[:, 4:5, :])
            nc.vector.tensor_copy(mrg[4][:, 0:1, :], loc8_pay[:, 0:1, :])

            # ---- merges + ood + transpose into map layout, batched by
            # engine so the five independent chains pipeline instead of
            # ping-ponging DVE->Scalar->DVE->PE with a sem hop each ----
            t8s = []
            for m in range(5):
                nc.vector.tensor_tensor(
                    mrg[m][:, 1:3, :],
                    rem8[:, :, m * 8 : (m + 1) * 8],
                    mask_sb[:],
                    op=ALU.add,
                )
            t8_all = smp.tile([128, 5, 8], F32, tag="t8", name="t8_all")
            for m in range(5):
                nc.vector.max(t8_all[:, m, :], mrg[m][:])
            oht_all = pp.tile([W, NCOL], F32, tag="u", name="oht", bufs=4)
            # x = (q2 - 2 t)/9 for all 5 merges in one DVE op, one 15-wide
            # sqrt, one reduce: ood = mean top-3 distance, f32
            x3 = smp.tile([128, 5, K_NN], F32, tag="x3", name="x3")
            nc.vector.scalar_tensor_tensor(
                x3[:], t8_all[:, :, 0:K_NN], -2.0 / 9.0, q2_sb[:],
                op0=ALU.mult, op1=ALU.add,
            )
            d3_all = smp.tile([128, 5, K_NN], F32, tag="d3", name="d3_all")
            nc.scalar.activation(d3_all[:], x3[:], AF.Sqrt)
            oodf = smp.tile([128, 5], F32, tag="oodf", name="oodf")
            nc.vector.reduce_sum(oodf[:], d3_all[:], axis=AX.X)
            for m in range(5):
                for blk in range(4):
                    nc.tensor.matmul(
                        oht_all[:, m * 4 + blk : m * 4 + blk + 1],
                        s4[:, blk, :],
                        oodf[:, m : m + 1],
                        start=True,
                        stop=True,
                    )
            nc.scalar.activation(ood_hT[:], oht_all[:], AF.Copy)

            if DEBUG:
                nc.sync.dma_start(
                    dbg_loc8[:, 0:5, :].rearrange("p a b -> p (a b)"),
                    loc8_pay[:].rearrange("p a b -> p (a b)"),
                )
                nc.sync.dma_start(dbg_oodht[:], ood_hT[:])

            # ---- bilinear upsample ----
            # each big matmul starts with an 8-col warmup slice so the bulk
            # runs at the mid p-state instead of cold; the copy/p2/out
            # chain is column-split so the two halves pipeline
            p1_ps = pp.tile([NCOL, OUT_W], F32, tag="v", name="p1", bufs=4)
            nc.tensor.matmul(
                p1_ps[:, 0:8], ood_hT[:], ac_sb[:, 0:8], start=True, stop=True
            )
            nc.tensor.matmul(
                p1_ps[:, 8:256], ood_hT[:], ac_sb[:, 8:256], start=True, stop=True
            )
            nc.tensor.matmul(
                p1_ps[:, 256:512], ood_hT[:], ac_sb[:, 256:512],
                start=True, stop=True,
            )
            p1_sb = sp.tile([NCOL, OUT_W], BF16)
            p2s, osbs = [], []
            for m2 in range(2):
                p2 = pp.tile([128, OUT_W], F32, tag="v", name=f"p2_{m2}", bufs=4)
                o_sb = smp.tile([128, OUT_W], BF16, tag="osb", name=f"osb{m2}")
                p2s.append(p2)
                osbs.append(o_sb)
            nc.scalar.activation(p1_sb[:, 0:256], p1_ps[:, 0:256], AF.Copy)
            nc.tensor.matmul(
                p2s[0][:, 0:8], art_sb[:, 0, :], p1_sb[:, 0:8],
                start=True, stop=True,
            )
            nc.tensor.matmul(
                p2s[0][:, 8:256], art_sb[:, 0, :], p1_sb[:, 8:256],
                start=True, stop=True,
            )
            nc.tensor.matmul(
                p2s[1][:, 0:256], art_sb[:, 1, :], p1_sb[:, 0:256],
                start=True, stop=True,
            )
            nc.scalar.activation(p1_sb[:, 256:512], p1_ps[:, 256:512], AF.Copy)
            nc.tensor.matmul(
                p2s[0][:, 256:512], art_sb[:, 0, :], p1_sb[:, 256:512],
                start=True, stop=True,
            )
            nc.tensor.matmul(
                p2s[1][:, 256:512], art_sb[:, 1, :], p1_sb[:, 256:512],
                start=True, stop=True,
            )
            # block 0 drains via ScalarE copies + SP-issued DMAs, block 1
            # via DVE copies + Pool-issued (SWDGE) DMAs -- parallel lanes
            nc.scalar.activation(osbs[0][:, 0:256], p2s[0][:, 0:256], AF.Copy)
            nc.sync.dma_start(out[0, :, 0:256], osbs[0][:, 0:256])
            with nc.allow_low_precision(reason="bf16 output map"):
                nc.vector.tensor_copy(osbs[1][:, 0:256], p2s[1][:, 0:256])
            nc.gpsimd.dma_start(out[1, :, 0:256], osbs[1][:, 0:256])
            nc.scalar.activation(osbs[0][:, 256:512], p2s[0][:, 256:512], AF.Copy)
            nc.sync.dma_start(out[0, :, 256:512], osbs[0][:, 256:512])
            with nc.allow_low_precision(reason="bf16 output map"):
                nc.vector.tensor_copy(osbs[1][:, 256:512], p2s[1][:, 256:512])
            nc.gpsimd.dma_start(out[1, :, 256:512], osbs[1][:, 256:512])

    nc.compile()
    return nc


def _bilinear_matrix(out_size: int, in_size: int) -> np.ndarray:
    """Half-pixel (align_corners=False) bilinear interpolation matrix
    [out_size, in_size]; edge-clamped, equivalent to jax.image.resize
    'bilinear' for integer upsampling."""
    A = np.zeros((out_size, in_size), dtype=np.float64)
    scale = in_size / out_size
    for i in range(out_size):
        s = (i + 0.5) * scale - 0.5
        j0 = int(np.floor(s))
        w = s - j0
        A[i, min(max(j0, 0), in_size - 1)] += 1.0 - w
        A[i, min(max(j0 + 1, 0), in_size - 1)] += w
    return A.astype(np.float32)


_NC_CACHE = None


def _get_nc():
    global _NC_CACHE
    if _NC_CACHE is None:
        _NC_CACHE = _build_program()
    return _NC_CACHE


def _slot_pack(db_half: np.ndarray):
    """Fold all column pairs of one db half [NHALF, 768]: per 1000-col
    group the slot layout is [500 a=(x0+x1)/2 | 500 b=(x0-x1)/2] over 764
    dims, with the matching xh = -(||x0||^2 +- ||x1||^2)/4 terms (over all
    768 dims, scaled 1/XS) as a 4-level fp8 split in rows 764..767."""
    n = db_half.shape[0]
    h = 0.5 * np.einsum("nd,nd->n", db_half, db_half)
    dbX = np.empty((n, 768), dtype=np.float32)
    xhX = np.empty(n, dtype=np.float32)
    for g in range(n // 1000):
        base = g * 1000
        p0 = db_half[base : base + 1000 : 2, :DX]
        p1 = db_half[base + 1 : base + 1000 : 2, :DX]
        h0 = h[base : base + 1000 : 2]
        h1 = h[base + 1 : base + 1000 : 2]
        dbX[base : base + 500, :DX] = 0.5 * (p0 + p1)
        dbX[base + 500 : base + 1000, :DX] = 0.5 * (p0 - p1)
        xhX[base : base + 500] = -0.5 * (h0 + h1)
        xhX[base + 500 : base + 1000] = -0.5 * (h0 - h1)
    r = (xhX / XS).astype(np.float32)
    for lv in range(4):
        q = r.astype(ml_dtypes.float8_e4m3).astype(np.float32)
        dbX[:, DX + lv] = q
        r = r - q
    return dbX


def make_in_maps(embeddings: np.ndarray, database: np.ndarray):
    embeddings = np.asarray(embeddings, dtype=np.float32)
    database = np.asarray(database, dtype=np.float32)

    q_all = embeddings.transpose(0, 2, 3, 1).reshape(B, H * W, D)
    A512 = _bilinear_matrix(OUT_W, W)                    # [512, 32]
    s4 = np.eye(128, dtype=np.float32).reshape(128, 4, W)
    ac = np.ascontiguousarray(A512.T).astype(ml_dtypes.bfloat16)  # [32, 512]

    db8_half = []
    for half in range(2):
        dbX = _slot_pack(database[half * NHALF : (half + 1) * NHALF])
        db8_half.append(
            np.ascontiguousarray(
                dbX.T.reshape(NKP, 2, 128, NHALF).transpose(2, 0, 1, 3)
            ).astype(ml_dtypes.float8_e4m3)
        )

    in_maps = []
    for c in range(N_CORES):
        b, half = divmod(c, 2)
        pos_rows = POS_ROWS[half]

        # queries of all 8 tiles in position order; the DR identity
        # block occupies cols 128:256 (kp0 ktile0 = I, rest 0)
        q = np.concatenate(
            [q_all[b, r0 * W : (r0 + 4) * W] for r0 in pos_rows]
        )                                                # [1024, 768]
        qX = np.zeros((QW, 768), dtype=np.float32)
        qX[0:128] = q[0:128]
        qX[0:128, DX:] = XS
        qX[128:256, 0:128] = np.eye(128, dtype=np.float32)
        qX[256:QW] = q[128:QTOT]
        qX[256:QW, DX:] = XS
        q8 = np.ascontiguousarray(
            qX.T.reshape(NKP, 2, 128, QW).transpose(2, 0, 1, 3)
        ).astype(ml_dtypes.float8_e4m3)                  # [128, 3, 2, 1152]
        q2 = np.einsum("qd,qd->q", q, q) / 9.0
        q2 = q2.reshape(NT, 128).T.astype(np.float32)      # [128, NT]
        q2m = np.ascontiguousarray(
            np.repeat(q2[:, MERGE_Q2COL, None], K_NN, axis=2)
        ).astype(np.float32)                               # [128, 5, 3]

        # gather mask: my own rank's block can never win a merge
        mask = np.zeros((128, 2, 8), dtype=np.float32)
        mask[:, c % 2, :] = NEG

        # art: j-columns [M_bnd, M_a, M_b, M_c, halo] x 4 rows each
        grow = [pos_rows[4 + t] + r for t in range(4) for r in range(4)] + [
            pos_rows[0] + r for r in range(4)
        ]
        rowsA = A512[half * 256 : (half + 1) * 256]      # [256, 32]
        art = np.zeros((NCOL, 2, 128), dtype=np.float32)
        for j, g in enumerate(grow):
            art[j, 0, :] = rowsA[0:128, g]
            art[j, 1, :] = rowsA[128:256, g]

        in_maps.append(
            {
                "db8": db8_half[half],
                "q8": q8,
                "q2": q2m,
                "s4": s4,
                "mask": mask,
                "art": art.astype(ml_dtypes.bfloat16),
                "ac": ac,
            }
        )
    return in_maps


def run_device(in_maps, **kwargs):
    nc = _get_nc()
    return bass_utils.run_bass_kernel_spmd(
        nc, in_maps, core_ids=list(range(N_CORES)), **kwargs
    )


def kernel(embeddings, database, k, out_h, out_w):
    assert int(k) == K_NN and int(out_h) == OUT_H and int(out_w) == OUT_W
    in_maps = make_in_maps(np.asarray(embeddings), np.asarray(database))
    res = run_device(in_maps)
    out = np.empty((B, 1, OUT_H, OUT_W), dtype=np.float32)
    for c in range(N_CORES):
        b, half = divmod(c, 2)
        o = np.asarray(res.results[c]["out"], dtype=np.float32)
        out[b, 0, half * 256 : half * 256 + 128] = o[0]
        out[b, 0, half * 256 + 128 : (half + 1) * 256] = o[1]
    return out


# revision 22
# speedup vs baseline: 1.0751x; 1.0032x over previous
"""Distributed kNN OOD-score kernel for 8 Trainium2 NeuronCores (v5).

Problem: for each of 4*32*32 query vectors (D=768), find the 3 nearest
database vectors (N=20000, squared-L2), average the 3 distances, and
bilinearly upsample the resulting [4,32,32] map to [4,1,512,512].

Sharding (v5, pair-split database): cores 2b and 2b+1 both work on batch
image b. The DATABASE is split between them (core half=0 streams entries
0..9999, half=1 streams 10000..19999 -- 7.7MB fp8 per core instead of
15.4MB replicated, which removes the DMA-starvation the v4 kernel hit),
and each core scores its half against ALL 1024 queries of the batch
(8 query tiles of 128). Per-tile top-8 candidates are then exchanged
within the pair by ONE small AllGather and merged (top-8 of own-24) to
exact top-3 over the full database.

Tile order per core: [P_bnd, P_a, P_b, P_c, M_bnd, M_a, M_b, M_c] where
P_* are the partner's map-row blocks and M_* this core's own. The gather
payload is positions 0-4 (everything the partner needs: its four blocks
+ my boundary block for the bilinear halo), so the collective fires at
~42us and its ~19us latency (15us rendezvous + 2x2.7us DRAM staging)
lands just as the 58us scoring stream ends. The halo ood block is
computed redundantly on both cores from the gathered candidates, so no
second exchange is needed.

Scoring per 1000-col unit (same fp8 DoubleRow scheme as v4, which is at
the PE fp8 peak): db column pairs are host-folded into a=(x0+x1)/2,
b=(x0-x1)/2 streams; 6 DR matmuls give u,v banks; ScalarE takes |v| to
SBUF fp8; one DR identity-matmul adds it back (u+|v| = max(t0,t1)
exactly); DVE max8 scans the 500 folded maxima. The cross products keep
764 of 768 dims; 4 freed contraction rows carry -||x||^2/2 as a 4-level
fp8 split (x4 stationary scale).

Epilogue: per-tile merges -> mean top-3 distance -> PE-transpose into
map layout -> one [32,20]x[32,512] interpolation matmul (p1) -> two
[20,128]x[20,512] matmuls (p2) -> 2x[128,512] output DMAs.
"""

import sys

if "/opt/trn_rl_repo" not in sys.path:
    sys.path.insert(0, "/opt/trn_rl_repo")

import numpy as np
import ml_dtypes

import concourse.bass as bass
import concourse.bacc as bacc
import concourse.mybir as mybir
import concourse.tile as tile
from concourse import bass_utils

# Problem shape (hardcoded per contract).
B, D, H, W = 4, 768, 32, 32
N = 20000
K_NN = 3
OUT_H = OUT_W = 512
N_CORES = 8

NHALF = N // 2        # db entries per core
N_SC = NHALF // 1000  # 10 units of 1000 cols per query tile
NT = 8                # query tiles per core (128 queries each)
QTOT = NT * 128       # 1024 queries scored per core
QW = QTOT + 128       # q8 width: +128 cols carrying the DR identity
QCOL = [0] + [128 * (p + 1) for p in range(1, NT)]  # q8 col of tile p
NKP = 3               # K pairs: 764 data dims + 4 xh rows = 3 * (2*128)
DX = 764              # cross-term dims (768 minus 4 freed for xh rows)
NCOL = 20             # ood columns entering the upsample (16 own + 4 halo)
XS = 4.0              # xh scale, baked into constant query rows
NEG = -1.0e9          # mask value killing own-rank gather blocks

F32 = mybir.dt.float32
BF16 = mybir.dt.bfloat16
FP8 = mybir.dt.float8e4
AX = mybir.AxisListType
AF = mybir.ActivationFunctionType
ALU = mybir.AluOpType
DR = mybir.MatmulPerfMode.DoubleRow

# Map-row start of each 4-row block, per half, in position order
# [P_bnd, P_a, P_b, P_c, M_bnd, M_a, M_b, M_c].
POS_ROWS = (
    [16, 20, 24, 28, 12, 8, 4, 0],   # half 0 (top, own rows 0-15)
    [12, 8, 4, 0, 16, 20, 24, 28],   # half 1 (bottom, own rows 16-31)
)

# Unit schedule: sc-major over payload positions 0-4 first (their last
# chunk-9 units run right after chunk 9 lands at ~25us, so the payload is
# complete at ~39us and the collective hides), then own tiles 5-7.
STRIPS = (
    [(sc, pos) for sc in range(N_SC) for pos in range(5)]
    + [(sc, pos) for pos in range(5, NT) for sc in range(N_SC)]
)
# merge index m: 0=M_bnd, 1=M_a, 2=M_b, 3=M_c, 4=halo(P_bnd)
# q2 column of the tile each merge scores
MERGE_Q2COL = [4, 5, 6, 7, 0]
DEBUG = False


def _build_program():
    nc = bacc.Bacc(
        "TRN2", target_bir_lowering=False, debug=False, num_devices=N_CORES
    )
    q8d = nc.dram_tensor("q8", [128, NKP, 2, QW], FP8, kind="ExternalInput").ap()
    db8d = nc.dram_tensor(
        "db8", [128, NKP, 2, NHALF], FP8, kind="ExternalInput"
    ).ap()
    q2d = nc.dram_tensor("q2", [128, 5, K_NN], F32, kind="ExternalInput").ap()
    s4d = nc.dram_tensor("s4", [128, 4, W], F32, kind="ExternalInput").ap()
    maskd = nc.dram_tensor("mask", [128, 5, 2, 8], F32, kind="ExternalInput").ap()
    artd = nc.dram_tensor("art", [NCOL, 2, 128], BF16, kind="ExternalInput").ap()
    acd = nc.dram_tensor("ac", [W, OUT_W], BF16, kind="ExternalInput").ap()
    out = nc.dram_tensor("out", [2, 128, OUT_W], BF16, kind="ExternalOutput").ap()
    if DEBUG:
        dbg_loc8 = nc.dram_tensor(
            "dbg_loc8", [128, NT, 8], F32, kind="ExternalOutput"
        ).ap()
        dbg_oodht = nc.dram_tensor(
            "dbg_oodht", [W, NCOL], BF16, kind="ExternalOutput"
        ).ap()

    with tile.TileContext(nc) as tc:
        with (
            tc.tile_pool(name="static", bufs=1) as sp,
            tc.tile_pool(name="db", bufs=N_SC * NKP) as dbp,
            tc.tile_pool(name="absv", bufs=5) as avp,
            tc.tile_pool(name="small", bufs=8) as smp,
            tc.tile_pool(name="psum", bufs=4, space="PSUM") as pp,
            tc.tile_pool(name="dram", bufs=1, space="DRAM") as dp,
        ):
            # ---- input DMAs, in first-need order. The SP sequencer holds
            # ~650ns per dma_start, so the count is kept low: 3 q8 pieces
            # (the DR identity rides in q8 cols 128:256), per-kp chunks for
            # sc0 only, whole-chunk DMAs for sc1-9, misc last. ----
            q8 = sp.tile([128, NKP, 2, QW], FP8)
            # pos0 queries first (first matmul), identity block second
            # (first drain, ~3 units later)
            nc.sync.dma_start(q8[:, :, :, 0:128], q8d[:, :, :, 0:128])
            dbt = {}  # sc -> [128, NKP, 2, 1000] tile
            db0 = dbp.tile([128, NKP, 2, 1000], FP8, tag="db", name="db0")
            nc.sync.dma_start(db0[:, 0, :, :], db8d[:, 0, :, 0:1000])
            nc.sync.dma_start(q8[:, :, :, 256:384], q8d[:, :, :, 256:384])
            nc.sync.dma_start(db0[:, 1, :, :], db8d[:, 1, :, 0:1000])
            nc.sync.dma_start(q8[:, :, :, 128:256], q8d[:, :, :, 128:256])
            nc.sync.dma_start(db0[:, 2, :, :], db8d[:, 2, :, 0:1000])
            dbt[0] = db0
            nc.sync.dma_start(q8[:, :, :, 384:768], q8d[:, :, :, 384:768])
            for sc in range(1, N_SC):
                t = dbp.tile([128, NKP, 2, 1000], FP8, tag="db", name=f"db{sc}")
                nc.sync.dma_start(
                    t[:], db8d[:, :, :, sc * 1000 : (sc + 1) * 1000]
                )
                dbt[sc] = t
                if sc == 4:
                    nc.sync.dma_start(q8[:, :, :, 768:QW], q8d[:, :, :, 768:QW])
            q2_sb = sp.tile([128, 5, K_NN], F32)
            nc.sync.dma_start(q2_sb[:], q2d[:])
            ident = q8[:, 0, :, 128:256]
            mask_sb = sp.tile([128, 5, 2, 8], F32)
            nc.sync.dma_start(mask_sb[:], maskd[:])
            s4 = sp.tile([128, 4, W], F32)
            nc.sync.dma_start(s4[:], s4d[:])
            art_sb = sp.tile([NCOL, 2, 128], BF16)
            nc.sync.dma_start(art_sb[:], artd[:])
            ac_sb = sp.tile([W, OUT_W], BF16)
            nc.sync.dma_start(ac_sb[:], acd[:])

            # ---- working state ----
            parts = [
                sp.tile([128, (N_SC + 1) * 8], F32, name=f"part{p}")
                for p in range(NT)
            ]
            loc8_pay = sp.tile([128, 5, 8], F32)       # payload top-8s, pos 0-4
            # merge staging: slot0 = own top-8, slots 1-2 = masked gather
            mrg_all = sp.tile([128, 5, 3, 8], F32)
            cc_in = dp.tile([128 * 40], F32)
            cc_out = dp.tile([2 * 128 * 40], F32)
            rem8 = sp.tile([128, 2, 40], F32)          # gathered payloads
            ood_hT = sp.tile([W, NCOL], BF16)

            def rhs(sc, kp, col, width):
                return dbt[sc][:, kp, :, col : col + width]

            pending = []

            def drain_one():
                u, absv, sw, part_ap = pending.pop(0)
                nc.tensor.matmul(
                    u[:, 0, 0:sw],
                    ident,
                    absv[:, :, 0:sw],
                    start=False,
                    stop=True,
                    perf_mode=DR,
                )
                nc.vector.max(part_ap, u[:, 0, 0:sw])

            def tile_epilogue(pos):
                m = pos - 4  # merge index for own tiles (pos 5,6,7 -> 1,2,3)
                if pos <= 4:
                    out8 = loc8_pay[:, pos, :]
                else:
                    out8 = mrg_all[:, m, 0, :]
                nw = (N_SC + 1) * 8 if pos == NT - 1 else N_SC * 8
                nc.vector.max(out8, parts[pos][:, 0:nw])
                if pos == 4:
                    # payload complete: pair-exchange positions 0-4
                    # (gpsimd-issued DMAs: 25ns sequencer hold vs 650 on SP)
                    nc.gpsimd.dma_start(cc_in[:], loc8_pay[:])
                    nc.gpsimd.collective_compute(
                        "AllGather",
                        ALU.bypass,
                        replica_groups=[[0, 1], [2, 3], [4, 5], [6, 7]],
                        ins=[cc_in.opt()],
                        outs=[cc_out.opt()],
                    )
                    nc.sync.dma_start(
                        rem8[:],
                        cc_out.rearrange("(r p f) -> p r f", r=2, p=128),
                    )

            # last strip index of each tile, and where to emit its epilogue
            # (>=3 units later so the pending drain has naturally passed it)
            last_idx = {}
            for i, (sc, pos) in enumerate(STRIPS):
                last_idx[pos] = i
            emit_at = {}
            for pos, idx in last_idx.items():
                emit_at.setdefault(min(idx + 2, len(STRIPS) - 1), []).append(pos)

            memset_count = 0
            for si, (sc, pos) in enumerate(STRIPS):
                c0 = QCOL[pos]
                lhsT = [
                    q8[:, kp, :, c0 : c0 + 128] for kp in range(NKP)
                ]
                last_split = si == len(STRIPS) - 1
                subs = [(0, 250), (250, 250)] if last_split else [(0, 500)]
                for hs, (s0, sw) in enumerate(subs):
                    u_ps = pp.tile([128, 1, 512], F32, tag="u", name="u", bufs=4)
                    v_ps = pp.tile([128, 1, 512], F32, tag="v", name="v", bufs=4)
                    for kp in range(NKP):
                        nc.tensor.matmul(
                            u_ps[:, 0, 0:sw],
                            lhsT[kp],
                            rhs(sc, kp, s0, sw),
                            start=(kp == 0),
                            stop=False,
                            perf_mode=DR,
                        )
                        nc.tensor.matmul(
                            v_ps[:, 0, 0:sw],
                            lhsT[kp],
                            rhs(sc, kp, 500 + s0, sw),
                            start=(kp == 0),
                            stop=(kp == NKP - 1),
                            perf_mode=DR,
                        )
                    absv = avp.tile([128, 2, 500], FP8, tag="absv", name="absv")
                    if memset_count < 5:
                        nc.gpsimd.memset(absv[:, 1, 0:500], 0.0)
                        memset_count += 1
                    nc.scalar.activation(
                        absv[:, 0, 0:sw], v_ps[:, 0, 0:sw], AF.Abs
                    )
                    pending.append(
                        (u_ps, absv, sw,
                         parts[pos][:, (sc + hs) * 8 :][:, 0:8])
                    )
                    while len(pending) > 2:
                        drain_one()
                for pos_done in emit_at.get(si, []):
                    if last_idx[pos_done] < si - 1:
                        tile_epilogue(pos_done)
                    else:
                        # too fresh (end of stream): flush pending first
                        while pending:
                            drain_one()
                        tile_epilogue(pos_done)
            while pending:
                drain_one()

            # own-top8 slots for the bnd and halo merges come from the payload
            nc.vector.tensor_copy(mrg_all[:, 0, 0, :], loc8_pay[:, 4, :])
            nc.vector.tensor_copy(mrg_all[:, 4, 0, :], loc8_pay[:, 0, :])

            # ---- merges: one batched mask-add, then top-8 per tile ----
            nc.vector.tensor_tensor(
                mrg_all[:, :, 1:3, :],
                rem8.rearrange("p r (j f) -> p j r f", f=8),
                mask_sb[:],
                op=ALU.add,
            )
            t8_all = smp.tile([128, 5, 8], F32, tag="t8", name="t8_all")
            for m in range(5):
                nc.vector.max(t8_all[:, m, :], mrg_all[:, m, :, :])
            # x = (q2 - 2 t)/9 for all 5 merges in one DVE op
            x3 = smp.tile([128, 5, K_NN], F32, tag="x3", name="x3")
            nc.vector.scalar_tensor_tensor(
                x3[:], t8_all[:, :, 0:K_NN], -2.0 / 9.0, q2_sb[:],
                op0=ALU.mult, op1=ALU.add,
            )
            # transpose into map layout BEFORE the sqrt: 20 tiny f32
            # matmuls, then ONE 60-wide sqrt and ONE 3-sum -> ood_hT
            xT_ps = pp.tile([W, 5, 4, K_NN], F32, tag="u", name="xT", bufs=4)
            for m in range(5):
                for blk in range(4):
                    nc.tensor.matmul(
                        xT_ps[:, m, blk, :],
                        s4[:, blk, :],
                        x3[:, m, :],
                        start=True,
                        stop=True,
                    )
            d3T = smp.tile([W, NCOL, K_NN], F32, tag="d3", name="d3T")
            nc.scalar.activation(
                d3T[:], xT_ps.rearrange("c m b k -> c (m b) k"), AF.Sqrt
            )
            with nc.allow_low_precision(
                reason="3-element sum rounded to bf16 for the upsample "
                "matmul; ~0.2% on a 2% tolerance"
            ):
                nc.vector.reduce_sum(ood_hT[:], d3T[:], axis=AX.X)

            if DEBUG:
                nc.sync.dma_start(
                    dbg_loc8[:, 0:5, :].rearrange("p a b -> p (a b)"),
                    loc8_pay[:].rearrange("p a b -> p (a b)"),
                )
                nc.sync.dma_start(dbg_oodht[:], ood_hT[:])

            # ---- bilinear upsample ----
            # each big matmul starts with an 8-col warmup slice so the bulk
            # runs at the mid p-state instead of cold; the copy/p2/out
            # chain is column-split so the two halves pipeline
            p1_ps = pp.tile([NCOL, OUT_W], F32, tag="v", name="p1", bufs=4)
            nc.tensor.matmul(
                p1_ps[:, 0:8], ood_hT[:], ac_sb[:, 0:8], start=True, stop=True
            )
            nc.tensor.matmul(
                p1_ps[:, 8:256], ood_hT[:], ac_sb[:, 8:256], start=True, stop=True
            )
            nc.tensor.matmul(
                p1_ps[:, 256:512], ood_hT[:], ac_sb[:, 256:512],
                start=True, stop=True,
            )
            p1_sb = sp.tile([NCOL, OUT_W], BF16)
            p2s, osbs = [], []
            for m2 in range(2):
                p2 = pp.tile([128, OUT_W], F32, tag="v", name=f"p2_{m2}", bufs=4)
                o_sb = smp.tile([128, OUT_W], BF16, tag="osb", name=f"osb{m2}")
                p2s.append(p2)
                osbs.append(o_sb)
            nc.scalar.activation(p1_sb[:, 0:256], p1_ps[:, 0:256], AF.Copy)
            with nc.allow_low_precision(reason="bf16 interp stage"):
                nc.vector.tensor_copy(p1_sb[:, 256:512], p1_ps[:, 256:512])
            nc.tensor.matmul(
                p2s[0][:, 0:8], art_sb[:, 0, :], p1_sb[:, 0:8],
                start=True, stop=True,
            )
            nc.tensor.matmul(
                p2s[0][:, 8:256], art_sb[:, 0, :], p1_sb[:, 8:256],
                start=True, stop=True,
            )
            nc.tensor.matmul(
                p2s[1][:, 0:256], art_sb[:, 1, :], p1_sb[:, 0:256],
                start=True, stop=True,
            )
            nc.tensor.matmul(
                p2s[0][:, 256:512], art_sb[:, 0, :], p1_sb[:, 256:512],
                start=True, stop=True,
            )
            nc.tensor.matmul(
                p2s[1][:, 256:512], art_sb[:, 1, :], p1_sb[:, 256:512],
                start=True, stop=True,
            )
            # block 0 drains via ScalarE copy + SP-issued DMA, block 1 via
            # DVE copy + Pool-issued (SWDGE) DMA -- fully parallel lanes
            nc.scalar.activation(osbs[0][:], p2s[0][:], AF.Copy)
            nc.sync.dma_start(out[0, :, :], osbs[0][:])
            with nc.allow_low_precision(reason="bf16 output map"):
                nc.vector.tensor_copy(osbs[1][:], p2s[1][:])
            nc.gpsimd.dma_start(out[1, :, :], osbs[1][:])

    nc.compile()
    return nc


def _bilinear_matrix(out_size: int, in_size: int) -> np.ndarray:
    """Half-pixel (align_corners=False) bilinear interpolation matrix
    [out_size, in_size]; edge-clamped, equivalent to jax.image.resize
    'bilinear' for integer upsampling."""
    A = np.zeros((out_size, in_size), dtype=np.float64)
    scale = in_size / out_size
    for i in range(out_size):
        s = (i + 0.5) * scale - 0.5
        j0 = int(np.floor(s))
        w = s - j0
        A[i, min(max(j0, 0), in_size - 1)] += 1.0 - w
        A[i, min(max(j0 + 1, 0), in_size - 1)] += w
    return A.astype(np.float32)


_NC_CACHE = None


def _get_nc():
    global _NC_CACHE
    if _NC_CACHE is None:
        _NC_CACHE = _build_program()
    return _NC_CACHE


def _slot_pack(db_half: np.ndarray):
    """Fold all column pairs of one db half [NHALF, 768]: per 1000-col
    group the slot layout is [500 a=(x0+x1)/2 | 500 b=(x0-x1)/2] over 764
    dims, with the matching xh = -(||x0||^2 +- ||x1||^2)/4 terms (over all
    768 dims, scaled 1/XS) as a 4-level fp8 split in rows 764..767."""
    n = db_half.shape[0]
    h = 0.5 * np.einsum("nd,nd->n", db_half, db_half)
    dbX = np.empty((n, 768), dtype=np.float32)
    xhX = np.empty(n, dtype=np.float32)
    for g in range(n // 1000):
        base = g * 1000
        p0 = db_half[base : base + 1000 : 2, :DX]
        p1 = db_half[base + 1 : base + 1000 : 2, :DX]
        h0 = h[base : base + 1000 : 2]
        h1 = h[base + 1 : base + 1000 : 2]
        dbX[base : base + 500, :DX] = 0.5 * (p0 + p1)
        dbX[base + 500 : base + 1000, :DX] = 0.5 * (p0 - p1)
        xhX[base : base + 500] = -0.5 * (h0 + h1)
        xhX[base + 500 : base + 1000] = -0.5 * (h0 - h1)
    r = (xhX / XS).astype(np.float32)
    for lv in range(4):
        q = r.astype(ml_dtypes.float8_e4m3).astype(np.float32)
        dbX[:, DX + lv] = q
        r = r - q
    return dbX


def make_in_maps(embeddings: np.ndarray, database: np.ndarray):
    embeddings = np.asarray(embeddings, dtype=np.float32)
    database = np.asarray(database, dtype=np.float32)

    q_all = embeddings.transpose(0, 2, 3, 1).reshape(B, H * W, D)
    A512 = _bilinear_matrix(OUT_W, W)                    # [512, 32]
    s4 = np.eye(128, dtype=np.float32).reshape(128, 4, W)
    ac = np.ascontiguousarray(A512.T).astype(ml_dtypes.bfloat16)  # [32, 512]

    db8_half = []
    for half in range(2):
        dbX = _slot_pack(database[half * NHALF : (half + 1) * NHALF])
        db8_half.append(
            np.ascontiguousarray(
                dbX.T.reshape(NKP, 2, 128, NHALF).transpose(2, 0, 1, 3)
            ).astype(ml_dtypes.float8_e4m3)
        )

    in_maps = []
    for c in range(N_CORES):
        b, half = divmod(c, 2)
        pos_rows = POS_ROWS[half]

        # queries of all 8 tiles in position order; the DR identity
        # block occupies cols 128:256 (kp0 ktile0 = I, rest 0)
        q = np.concatenate(
            [q_all[b, r0 * W : (r0 + 4) * W] for r0 in pos_rows]
        )                                                # [1024, 768]
        qX = np.zeros((QW, 768), dtype=np.float32)
        qX[0:128] = q[0:128]
        qX[0:128, DX:] = XS
        qX[128:256, 0:128] = np.eye(128, dtype=np.float32)
        qX[256:QW] = q[128:QTOT]
        qX[256:QW, DX:] = XS
        q8 = np.ascontiguousarray(
            qX.T.reshape(NKP, 2, 128, QW).transpose(2, 0, 1, 3)
        ).astype(ml_dtypes.float8_e4m3)                  # [128, 3, 2, 1152]
        q2 = np.einsum("qd,qd->q", q, q) / 9.0
        q2 = q2.reshape(NT, 128).T.astype(np.float32)      # [128, NT]
        q2m = np.ascontiguousarray(
            np.repeat(q2[:, MERGE_Q2COL, None], K_NN, axis=2)
        ).astype(np.float32)                               # [128, 5, 3]

        # gather mask: my own rank's block can never win a merge
        mask = np.zeros((128, 5, 2, 8), dtype=np.float32)
        mask[:, :, c % 2, :] = NEG

        # art: j-columns [M_bnd, M_a, M_b, M_c, halo] x 4 rows each
        grow = [pos_rows[4 + t] + r for t in range(4) for r in range(4)] + [
            pos_rows[0] + r for r in range(4)
        ]
        rowsA = A512[half * 256 : (half + 1) * 256]      # [256, 32]
        art = np.zeros((NCOL, 2, 128), dtype=np.float32)
        for j, g in enumerate(grow):
            art[j, 0, :] = rowsA[0:128, g]
            art[j, 1, :] = rowsA[128:256, g]

        in_maps.append(
            {
                "db8": db8_half[half],
                "q8": q8,
                "q2": q2m,
                "s4": s4,
                "mask": mask,
                "art": art.astype(ml_dtypes.bfloat16),
                "ac": ac,
            }
        )
    return in_maps


def run_device(in_maps, **kwargs):
    nc = _get_nc()
    return bass_utils.run_bass_kernel_spmd(
        nc, in_maps, core_ids=list(range(N_CORES)), **kwargs
    )


def kernel(embeddings, database, k, out_h, out_w):
    assert int(k) == K_NN and int(out_h) == OUT_H and int(out_w) == OUT_W
    in_maps = make_in_maps(np.asarray(embeddings), np.asarray(database))
    res = run_device(in_maps)
    out = np.empty((B, 1, OUT_H, OUT_W), dtype=np.float32)
    for c in range(N_CORES):
        b, half = divmod(c, 2)
        o = np.asarray(res.results[c]["out"], dtype=np.float32)
        out[b, 0, half * 256 : half * 256 + 128] = o[0]
        out[b, 0, half * 256 + 128 : (half + 1) * 256] = o[1]
    return out
